# revision 13
# baseline (speedup 1.0000x reference)
"""CPG network kernel for 8 Trainium2 NeuronCores.

Sharding (tensor-parallel, 2 AllGathers total):
  in-MLP:  L0 row-sharded (512 rows/core) -> h0_c; L1 column-sharded over
           input (core's h0_c) -> partial h1 (4096) -> AllGather partials
           -> local sum(+bias as 9th slot)+relu (h1 replicated)
  L2 row-sharded (264 rows/core) -> AllGather params (2112, replicated)
  CPG RK4 (32 oscillators) replicated on every core
  out-MLP: L0 replicated (full g0 on every core, out_W0 is only 0.5MB),
           L1 row-sharded (core computes complete g1[256c:256c+256]),
           L2 column-sharded over g1 -> each core returns a partial out[1024];
           host sums the 8 partials and adds out_b2.

GEMV on PE: lhsT = host-pretransposed weight tile [k=128, m<=128] stationary,
rhs = activation column [k<=128, 1] moving, accumulate over k-chunks in PSUM.
Weights stream HBM->SBUF in ~1-2MB blocks, fp32 end to end. Biases are folded
into the GEMVs where possible (L0 via x_aug[2049]=1, outL0 via a bias row with
rhs=1); sin/cos use ACT Sin with explicit round-to-nearest range reduction.
"""
import os
import sys

import numpy as np

for _p in ("/opt/trn_rl_repo", "/root/.axon_site/_ro/trn_rl_repo"):
    if os.path.isdir(_p) and _p not in sys.path:
        sys.path.append(_p)

import concourse.bacc as bacc
import concourse.mybir as mybir
import concourse.tile as tile
from concourse.tile import add_dep_helper
from concourse.bass_utils import run_bass_kernel_spmd

F32 = mybir.dt.float32
I32 = mybir.dt.int32
AF = mybir.ActivationFunctionType
OP = mybir.AluOpType

NCORES = 8
N = 32
TWO_PI = float(2.0 * np.pi)
INV_2PI = float(1.0 / (2.0 * np.pi))

_PROG = None
_PERM = None


def _params_perm():
    global _PERM
    if _PERM is None:
        g = np.arange(2112)
        i, c = g // 66, g % 66
        _PERM = np.where(
            c == 0, i,
            np.where(c == 1, 32 + i,
                     np.where(c < 34, 64 + 32 * i + (c - 2),
                              1088 + 32 * i + (c - 34)))).astype(np.int64)
    return _PERM


def _w2_perm(inp):
    return np.asarray(inp["in_W2"], np.float32)[_params_perm()]


def _b2_perm(inp):
    return np.asarray(inp["in_b2"], np.float32)[_params_perm()]


def _cpg_f_eval(nc, sb, ps, y, k, cst, sinr_e, acw_e):
    """k = f(y) for the CPG ODE. y, k: [32,3] tiles (cols a, a_dot, ph).

    sinr_e/acw_e: persistent [N, N+1] tiles; column N holds (1.0, ifr[i]) so
    the fused multiply-accumulate yields phase_dots = ifr + sum_j(...) in one
    instruction."""
    eye, ones, cw, pb, ia, ifr = cst
    # a, ph -> rows (PE transpose via identity), then broadcast to [32,32]
    arow_ps = ps.tile([1, N], F32, tag="rowps")
    phrow_ps = ps.tile([1, N], F32, tag="rowps")
    nc.tensor.matmul(arow_ps[:], y[:, 0:1], eye, start=True, stop=True)
    nc.tensor.matmul(phrow_ps[:], y[:, 2:3], eye, start=True, stop=True)
    arow = sb.tile([1, N], F32, tag="arow")
    phrow = sb.tile([1, N], F32, tag="phrow")
    nc.vector.tensor_copy(arow[:], arow_ps[:])
    nc.vector.tensor_copy(phrow[:], phrow_ps[:])
    A_ps = ps.tile([N, N], F32, tag="bcps")
    P_ps = ps.tile([N, N], F32, tag="bcps")
    nc.tensor.matmul(A_ps[:], ones, arow[:], start=True, stop=True)
    nc.tensor.matmul(P_ps[:], ones, phrow[:], start=True, stop=True)
    # arg[i,j] = ph_j - ph_i - pb[i,j], range-reduced into [-pi, pi]
    arg = sb.tile([N, N], F32, tag="arg")
    nc.vector.scalar_tensor_tensor(
        arg[:], P_ps[:], y[:, 2:3], pb[:], OP.subtract, OP.subtract)
    ri = sb.tile([N, N], I32, tag="ri")
    nc.vector.tensor_scalar(ri[:], arg[:], INV_2PI, None, OP.mult)
    nf = sb.tile([N, N], F32, tag="nf")
    nc.vector.tensor_copy(nf[:], ri[:])
    red = sb.tile([N, N], F32, tag="red")
    nc.vector.scalar_tensor_tensor(red[:], nf[:], -TWO_PI, arg[:], OP.mult, OP.add)
    nc.scalar.activation(sinr_e[:, 0:N], red[:], AF.Sin)
    # phase_dots = sum_j a_j * cw[i,j] * sin(arg[i,j])  + ifr via column N
    nc.vector.tensor_tensor(acw_e[:, 0:N], A_ps[:], cw[:], OP.mult)
    junk = sb.tile([N, N + 1], F32, tag="junk")
    nc.vector.scalar_tensor_tensor(
        junk[:], sinr_e[:], 0.0, acw_e[:], OP.bypass, OP.mult, accum_out=k[:, 1:2]
    )
    # k[:,0] = a_dot;  k[:,2] = 1000*(250*(ia - a) - a_dot)
    nc.vector.tensor_copy(k[:, 0:1], y[:, 1:2])
    t2 = sb.tile([N, 1], F32, tag="t2")
    nc.vector.tensor_scalar(t2[:], y[:, 0:1], ia[:], -250.0, OP.subtract, OP.mult)
    nc.vector.tensor_scalar(k[:, 2:3], t2[:], y[:, 1:2], 1000.0, OP.subtract, OP.mult)


def _range_reduce(nc, sb, src_col, extra_quarter_turn, tagp):
    """r = src - 2*pi*round((src + off)/(2*pi)); off = pi/2 when
    extra_quarter_turn so that r + pi/2 lands in [-pi, pi]."""
    ri = sb.tile([N, 1], I32, tag=tagp + "ri")
    if extra_quarter_turn:
        nc.vector.tensor_scalar(ri[:], src_col, INV_2PI, 0.25, OP.mult, OP.add)
    else:
        nc.vector.tensor_scalar(ri[:], src_col, INV_2PI, None, OP.mult)
    nf = sb.tile([N, 1], F32, tag=tagp + "nf")
    nc.vector.tensor_copy(nf[:], ri[:])
    r = sb.tile([N, 1], F32, tag=tagp + "r")
    nc.vector.scalar_tensor_tensor(r[:], nf[:], -TWO_PI, src_col, OP.mult, OP.add)
    return r


def _build():
    nc = bacc.Bacc(None, target_bir_lowering=False, num_devices=NCORES)

    # ---- I/O declarations (per-core shards, host-prepared layouts) ----
    m128_d = nc.dram_tensor("misc128", [128, 61], F32, kind="ExternalInput")
    m32_d = nc.dram_tensor("misc32", [N, 165], F32, kind="ExternalInput")
    m1_d = nc.dram_tensor("misc1", [1, 361], F32, kind="ExternalInput")
    w0_d = [nc.dram_tensor(f"w0b{j}", [128, 17 * 128], F32, kind="ExternalInput") for j in range(4)]
    w1_d = [nc.dram_tensor(f"w1b{j}", [128, 4 * 1024], F32, kind="ExternalInput") for j in range(4)]
    w2_d = [nc.dram_tensor(f"w2r{g}", [128, 8 * 264], F32, kind="ExternalInput")
            for g in range(4)]
    w3_d = nc.dram_tensor("w3t", [65, 2048], F32, kind="ExternalInput")
    w4_d = nc.dram_tensor("w4t", [128, 16 * 256], F32, kind="ExternalInput")
    w5_d = nc.dram_tensor("w5t", [128, 2 * 1024], F32, kind="ExternalInput")
    stout_d = nc.dram_tensor("state_out", [N, 3], F32, kind="ExternalOutput")
    out_d = nc.dram_tensor("out_part", [128, 8], F32, kind="ExternalOutput")

    with tile.TileContext(nc) as tc:
        with (
            tc.tile_pool(name="w", bufs=1) as wp,
            tc.tile_pool(name="act", bufs=1) as ap,
            tc.tile_pool(name="tmp", bufs=2) as sb,
            tc.tile_pool(name="ps", bufs=2, space="PSUM") as ps,
            tc.tile_pool(name="dram", bufs=1, space="DRAM") as dp,
        ):
            # ---- packed constant loads (3 DMAs on the scalar ring) ----
            m128 = ap.tile([128, 61], F32, tag="m128")
            nc.scalar.dma_start(m128[:], m128_d[:])
            m32 = ap.tile([N, 165], F32, tag="m32")
            nc.scalar.dma_start(m32[:], m32_d[:])
            m1 = ap.tile([1, 361], F32, tag="m1")
            nc.scalar.dma_start(m1[:], m1_d[:])
            xin = m128[:, 0:17]
            hc = m128[0:N, 17:24]
            b2t = m128[:, 24:27]
            b3t = m128[:, 27:29]
            b1f = m128[:, 29:61]
            eye = m32[:, 0:32]
            y1 = m32[:, 32:35]
            sel65a = m32[:, 35:100]
            sel65b = m32[:, 100:165]
            ones = m1[:, 0:32]
            e64 = m1[:, 32:97]
            b2row = m1[:, 97:361]
            warm = ap.tile([1, 1], F32, tag="warm")
            nc.scalar.activation(warm[:], ones[0:1, 0:1], AF.Sin)

            # ---- weight streams (sync = HWDGE ring A, consumption order) ----
            w0 = [wp.tile([128, 17 * 128], F32, tag=f"w0_{j}", name=f"w0_{j}") for j in range(4)]
            for j in range(4):
                nc.sync.dma_start(w0[j][:], w0_d[j][:])
            w1 = [wp.tile([128, 4 * 1024], F32, tag=f"w1_{j}", name=f"w1_{j}") for j in range(4)]
            for j in range(4):
                nc.sync.dma_start(w1[j][:], w1_d[j][:])
            w2 = [wp.tile([128, 8 * 264], F32, tag=f"w2_{g}", name=f"w2_{g}")
                  for g in range(4)]
            for g in range(4):
                nc.sync.dma_start(w2[g][:], w2_d[g][:])
            w3t = wp.tile([65, 2048], F32, tag="w3t")
            nc.sync.dma_start(w3t[:], w3_d[:])
            w4t = wp.tile([128, 16 * 256], F32, tag="w4t")
            w5t = wp.tile([128, 2 * 1024], F32, tag="w5t")

            # ---- L0: h0 = relu(W0 @ [x; t; 1]) (bias folded), 512 rows ----
            psum0 = ps.tile([128, 4], F32, tag="mm")
            for mj in range(4):
                for kc in range(17):
                    nc.tensor.matmul(
                        psum0[:, mj:mj + 1],
                        w0[mj][:, kc * 128:(kc + 1) * 128],
                        xin[:, kc:kc + 1],
                        start=(kc == 0), stop=(kc == 16),
                    )
            h0 = ap.tile([128, 4], F32, tag="h0")
            nc.vector.tensor_scalar_max(h0[:], psum0[:], 0.0)

            # ---- L1 partial: W1[:, cols_c] @ h0_c  (4096 partials) ----
            psum1 = ps.tile([128, 32], F32, tag="mm")
            for mj in range(32):
                blk, ml = mj // 8, (mj % 8) * 128
                for kc in range(4):
                    nc.tensor.matmul(
                        psum1[:, mj:mj + 1],
                        w1[blk][:, kc * 1024 + ml: kc * 1024 + ml + 128],
                        h0[:, kc:kc + 1],
                        start=(kc == 0), stop=(kc == 3),
                    )
            h1p = ap.tile([128, 32], F32, tag="h1p")
            nc.vector.tensor_copy(h1p[:], psum1[:])

            # ---- AllGather partials; local reduce (+bias as 9th slot) ----
            bounce1_in = dp.tile([128, 32], F32, tag="bc1i")
            bounce1_out = dp.tile([NCORES, 128, 32], F32, tag="bc1o")
            b1w = nc.scalar.dma_start(bounce1_in[:], h1p[:])
            # Out-MLP L1/L2 weights are needed only after the CPG; gating
            # them on the bounce write keeps the first 35us of HBM bandwidth
            # exclusively for the critical W0/W1 stream, then streams them
            # during the AllGather/L2 window.
            dw4 = nc.scalar.dma_start(w4t[:], w4_d[:])
            dw5 = nc.scalar.dma_start(w5t[:], w5_d[:])
            add_dep_helper(dw4.ins, b1w.ins, reason="defer w4t behind h1p bounce")
            add_dep_helper(dw5.ins, b1w.ins, reason="defer w5t behind h1p bounce")
            nc.gpsimd.collective_compute(
                "AllGather", OP.bypass, replica_groups=[list(range(NCORES))],
                ins=[bounce1_in.opt()], outs=[bounce1_out.opt()],
            )
            p8 = ap.tile([128, 9 * 32], F32, tag="p8")
            nc.scalar.dma_start(
                p8[:, 0:256].rearrange("p (r j) -> p r j", r=NCORES),
                bounce1_out[:].rearrange("r p j -> p r j"),
            )
            nc.vector.tensor_copy(p8[:, 256:288], b1f)
            h1s = ap.tile([128, 32], F32, tag="h1s")
            nc.vector.tensor_reduce(
                h1s[:], p8[:].rearrange("p (r j) -> p j r", r=9),
                axis=mybir.AxisListType.X, op=OP.add,
            )
            h1 = ap.tile([128, 32], F32, tag="h1")
            nc.vector.tensor_scalar_max(h1[:], h1s[:], 0.0)

            # ---- L2: params chunk = W2_c @ h1 + b2_c (264 rows).
            # h1 columns are the STATIONARY operand (1-column LDWEIGHTS is
            # ~free) and the weight streams as the moving operand, so this
            # layer costs 32 matmuls instead of 96 LDW-bound ones. The
            # [1,264] row output feeds the AllGather bounce directly. ----
            psum2r = ps.tile([1, 264], F32, tag="mmrow")
            for kc in range(32):
                nc.tensor.matmul(
                    psum2r[:],
                    h1[:, kc:kc + 1],
                    w2[kc // 8][:, (kc % 8) * 264:(kc % 8 + 1) * 264],
                    start=(kc == 0), stop=(kc == 31),
                )
            params_row = ap.tile([1, 264], F32, tag="params")
            nc.vector.tensor_add(params_row[:], psum2r[:], b2row)

            # ---- AllGather params -> replicated [ia, ifr, cw, pb] ----
            bounce2_in = dp.tile([1, 264], F32, tag="bc2i")
            bounce2_out = dp.tile([1, NCORES * 264], F32, tag="bc2o")
            nc.scalar.dma_start(bounce2_in[:], params_row[:])
            nc.gpsimd.collective_compute(
                "AllGather", OP.bypass, replica_groups=[list(range(NCORES))],
                ins=[bounce2_in.opt()], outs=[bounce2_out.opt()],
            )
            # host permuted W2/b2 so the gathered params are exactly a
            # row-major [32, 66] = [ia | ifr | cw row | pb row] tile
            pt = ap.tile([N, 66], F32, tag="pt")
            nc.scalar.dma_start(
                pt[:], bounce2_out[:].rearrange("o (i c) -> (o i) c", c=66))
            ia = pt[:, 0:1]
            ifr = pt[:, 1:2]
            cw = pt[:, 2:34]
            pb = pt[:, 34:66]

            # ---- CPG RK4 (3/8 rule), replicated.  Stage temps are arranged
            # so only ONE DVE op sits serially between f-evals. ----
            cst = (eye, ones, cw, pb, ia, ifr)
            h_c = hc[:, 0:1]
            h3_c = hc[:, 1:2]
            nh3_c = hc[:, 2:3]
            nh_c = hc[:, 3:4]
            h8_c = hc[:, 4:5]
            h38_c = hc[:, 5:6]
            k1 = ap.tile([N, 3], F32, tag="k1")
            k2 = ap.tile([N, 3], F32, tag="k2")
            k3 = ap.tile([N, 3], F32, tag="k3")
            k4 = ap.tile([N, 3], F32, tag="k4")
            sinr_es = [ap.tile([N, N + 1], F32, tag=f"sinr{e}", name=f"sinr{e}")
                       for e in range(4)]
            acw_es = [ap.tile([N, N + 1], F32, tag=f"acw{e}", name=f"acw{e}")
                      for e in range(4)]
            for e in range(4):
                nc.vector.memset(sinr_es[e][:, N:N + 1], 1.0)
                nc.vector.tensor_copy(acw_es[e][:, N:N + 1], ifr)
            _cpg_f_eval(nc, sb, ps, y1, k1, cst, sinr_es[0], acw_es[0])
            y2 = ap.tile([N, 3], F32, tag="y2")
            nc.vector.scalar_tensor_tensor(y2[:], k1[:], h3_c, y1, OP.mult, OP.add)
            # temps that only need k1 (overlap with eval 2)
            t13 = ap.tile([N, 3], F32, tag="t13")
            t15 = ap.tile([N, 3], F32, tag="t15")
            acc1 = ap.tile([N, 3], F32, tag="acc1")
            nc.vector.scalar_tensor_tensor(t13[:], k1[:], nh3_c, y1, OP.mult, OP.add)
            nc.vector.scalar_tensor_tensor(t15[:], k1[:], h_c, y1, OP.mult, OP.add)
            nc.vector.scalar_tensor_tensor(acc1[:], k1[:], h8_c, y1, OP.mult, OP.add)
            _cpg_f_eval(nc, sb, ps, y2, k2, cst, sinr_es[1], acw_es[1])
            y3 = ap.tile([N, 3], F32, tag="y3")
            nc.vector.scalar_tensor_tensor(y3[:], k2[:], h_c, t13[:], OP.mult, OP.add)
            t14 = ap.tile([N, 3], F32, tag="t14")
            acc2 = ap.tile([N, 3], F32, tag="acc2")
            nc.vector.scalar_tensor_tensor(t14[:], k2[:], nh_c, t15[:], OP.mult, OP.add)
            nc.vector.scalar_tensor_tensor(acc2[:], k2[:], h38_c, acc1[:], OP.mult, OP.add)
            _cpg_f_eval(nc, sb, ps, y3, k3, cst, sinr_es[2], acw_es[2])
            y4 = ap.tile([N, 3], F32, tag="y4")
            nc.vector.scalar_tensor_tensor(y4[:], k3[:], h_c, t14[:], OP.mult, OP.add)
            acc3 = ap.tile([N, 3], F32, tag="acc3")
            nc.vector.scalar_tensor_tensor(acc3[:], k3[:], h38_c, acc2[:], OP.mult, OP.add)
            _cpg_f_eval(nc, sb, ps, y4, k4, cst, sinr_es[3], acw_es[3])
            ynew = ap.tile([N, 3], F32, tag="ynew")
            nc.vector.scalar_tensor_tensor(ynew[:], k4[:], h8_c, acc3[:], OP.mult, OP.add)
            nc.scalar.dma_start(stout_d[:], ynew[:])

            # cpg_out = [a*cos(ph), a*sin(ph)]; reference reads ph from
            # new_state[n:2n], i.e. column 1 of ynew.
            rs = _range_reduce(nc, sb, ynew[:, 1:2], False, "rs")
            rc = _range_reduce(nc, sb, ynew[:, 1:2], True, "rc")
            sin_ph = ap.tile([N, 1], F32, tag="sinph")
            cos_ph = ap.tile([N, 1], F32, tag="cosph")
            nc.scalar.activation(sin_ph[:], rs[:], AF.Sin)
            nc.scalar.activation(cos_ph[:], rc[:], AF.Sin, bias=hc[:, 6:7])
            ac_col = ap.tile([N, 1], F32, tag="accol")
            as_col = ap.tile([N, 1], F32, tag="ascol")
            nc.vector.tensor_mul(ac_col[:], ynew[:, 0:1], cos_ph[:])
            nc.vector.tensor_mul(as_col[:], ynew[:, 0:1], sin_ph[:])

            # pack cpg_out (+bias 1.0) into a [65,1] column via PE selects
            cpg_ps = ps.tile([65, 1], F32, tag="mm")
            nc.tensor.matmul(cpg_ps[:], sel65a, ac_col[:], start=True, stop=False)
            nc.tensor.matmul(cpg_ps[:], sel65b, as_col[:], start=False, stop=False)
            nc.tensor.matmul(cpg_ps[:], e64, ones[0:1, 0:1], start=False, stop=True)
            cpg65 = ap.tile([65, 1], F32, tag="cpg65")
            nc.vector.tensor_copy(cpg65[:], cpg_ps[:])

            # ---- out-MLP L0 (replicated): g0 = relu(W0o @ cpg_out + b0o) ----
            psum3 = ps.tile([128, 16], F32, tag="mm")
            for mj in range(16):
                nc.tensor.matmul(psum3[:, mj:mj + 1],
                                 w3t[:, mj * 128:(mj + 1) * 128], cpg65[:],
                                 start=True, stop=True)
            g0 = ap.tile([128, 16], F32, tag="g0")
            nc.vector.tensor_scalar_max(g0[:], psum3[:], 0.0)

            # ---- out-MLP L1 (row-sharded): g1_c = relu(W1o_c @ g0 + b1o_c) ----
            psum4 = ps.tile([128, 2], F32, tag="mm")
            for mj in range(2):
                for kc in range(16):
                    nc.tensor.matmul(
                        psum4[:, mj:mj + 1],
                        w4t[:, kc * 256 + mj * 128: kc * 256 + mj * 128 + 128],
                        g0[:, kc:kc + 1],
                        start=(kc == 0), stop=(kc == 15),
                    )
            g1s = ap.tile([128, 2], F32, tag="g1s")
            nc.vector.tensor_add(g1s[:], psum4[:], b3t)
            g1c = ap.tile([128, 2], F32, tag="g1c")
            nc.vector.tensor_scalar_max(g1c[:], g1s[:], 0.0)

            # ---- out-MLP L2 (column-sharded): partial out[1024] ----
            psum5 = ps.tile([128, 8], F32, tag="mm")
            for mj in range(8):
                for kc in range(2):
                    nc.tensor.matmul(
                        psum5[:, mj:mj + 1],
                        w5t[:, kc * 1024 + mj * 128: kc * 1024 + mj * 128 + 128],
                        g1c[:, kc:kc + 1],
                        start=(kc == 0), stop=(kc == 1),
                    )
            out_sb = ap.tile([128, 8], F32, tag="outsb")
            nc.vector.tensor_copy(out_sb[:], psum5[:])
            nc.scalar.dma_start(out_d[:], out_sb[:])

    nc.compile()
    return nc


def _prep_core(c, inp):
    t = float(np.asarray(inp["timestep"], np.float32))
    x = np.asarray(inp["x"], np.float32)
    state = np.asarray(inp["state"], np.float32)

    x_aug = np.zeros(2176, np.float32)
    x_aug[:2048] = x
    x_aug[2048] = t
    x_aug[2049] = 1.0
    m = {}
    m128 = np.zeros((128, 61), np.float32)
    m128[:, 0:17] = x_aug.reshape(17, 128).T
    m128[:, 17:24] = np.array([t, t / 3, -t / 3, -t, t / 8, 3 * t / 8, np.pi / 2],
                              np.float32)[None, :]
    m32 = np.zeros((N, 165), np.float32)
    m32[:, 0:32] = np.eye(N, dtype=np.float32)
    m32[:, 32:35] = state.reshape(3, N).T
    m32[:, 35:100][:, 0:32] = np.eye(N, dtype=np.float32)
    m32[:, 100:165][:, 32:64] = np.eye(N, dtype=np.float32)
    m1 = np.zeros((1, 361), np.float32)
    m1[0, 0:32] = 1.0
    m1[0, 96] = 1.0
    m["misc128"] = m128
    m["misc32"] = m32
    m["misc1"] = m1

    R0 = slice(512 * c, 512 * c + 512)
    W0e = np.zeros((512, 2176), np.float32)
    W0e[:, :2049] = inp["in_W0"][R0]
    W0e[:, 2049] = inp["in_b0"][R0]
    for j in range(4):
        blk = W0e[128 * j:128 * (j + 1)]
        m[f"w0b{j}"] = np.ascontiguousarray(
            blk.reshape(128, 17, 128).transpose(2, 1, 0).reshape(128, 17 * 128))

    W1c = inp["in_W1"][:, 512 * c:512 * (c + 1)]
    for j in range(4):
        blk = W1c[1024 * j:1024 * (j + 1)]
        m[f"w1b{j}"] = np.ascontiguousarray(
            blk.reshape(1024, 4, 128).transpose(2, 1, 0).reshape(128, 4 * 1024))
    m128[:, 29:61] = inp["in_b1"].reshape(32, 128).T

    R2 = slice(264 * c, 264 * (c + 1))
    W2c = _w2_perm(inp)[R2]
    b2c = _b2_perm(inp)[R2]
    # moving-operand layout: w2r[p, kc*264 + m] = W2c[m, kc*128 + p]
    w2r = W2c.T.reshape(32, 128, 264).transpose(1, 0, 2).reshape(128, 32 * 264)
    for g in range(4):
        m[f"w2r{g}"] = np.ascontiguousarray(w2r[:, g * 8 * 264:(g + 1) * 8 * 264])
    m1[0, 97:361] = b2c

    m["w3t"] = np.ascontiguousarray(np.concatenate(
        [inp["out_W0"][:, 0:32].T, inp["out_W0"][:, 32:64].T,
         np.asarray(inp["out_b0"], np.float32)[None, :]], axis=0))

    W1oc = inp["out_W1"][256 * c:256 * (c + 1), :]
    m["w4t"] = np.ascontiguousarray(
        W1oc.reshape(256, 16, 128).transpose(2, 1, 0).reshape(128, 16 * 256))
    m128[:, 27:29] = np.asarray(
        inp["out_b1"], np.float32)[256 * c:256 * (c + 1)].reshape(2, 128).T

    W2oc = inp["out_W2"][:, 256 * c:256 * (c + 1)]
    m["w5t"] = np.ascontiguousarray(
        W2oc.reshape(1024, 2, 128).transpose(2, 1, 0).reshape(128, 2 * 1024))
    return m


def kernel(**inputs):
    global _PROG
    if _PROG is None:
        _PROG = _build()
    in_maps = [_prep_core(c, inputs) for c in range(NCORES)]
    res = run_bass_kernel_spmd(_PROG, in_maps, core_ids=list(range(NCORES)))
    state_out = np.ascontiguousarray(res.results[0]["state_out"].T).reshape(96)
    acc = res.results[0]["out_part"].astype(np.float32).copy()
    for c in range(1, NCORES):
        acc += res.results[c]["out_part"]
    out = np.ascontiguousarray(acc.T).reshape(1024) + np.asarray(inputs["out_b2"], np.float32)
    return (state_out.astype(np.float32), out.astype(np.float32))


# revision 19
# speedup vs baseline: 1.3167x; 1.3167x over previous
"""CPG network kernel for 8 Trainium2 NeuronCores.

Sharding (tensor-parallel, 2 AllGathers total):
  in-MLP:  L0 row-sharded (512 rows/core) -> h0_c; L1 column-sharded over
           input (core's h0_c) -> partial h1 (4096) -> AllGather partials
           -> local sum(+bias as 9th slot)+relu (h1 replicated)
  L2 row-sharded (264 rows/core) -> AllGather params (2112, replicated)
  CPG RK4 (32 oscillators) replicated on every core
  out-MLP: L0 replicated (full g0 on every core, out_W0 is only 0.5MB),
           L1 row-sharded (core computes complete g1[256c:256c+256]),
           L2 column-sharded over g1 -> each core returns a partial out[1024];
           host sums the 8 partials and adds out_b2.

GEMV on PE: lhsT = host-pretransposed weight tile [k=128, m<=128] stationary,
rhs = activation column [k<=128, 1] moving, accumulate over k-chunks in PSUM.
Weights stream HBM->SBUF in ~1-2MB blocks, fp32 end to end. Biases are folded
into the GEMVs where possible (L0 via x_aug[2049]=1, outL0 via a bias row with
rhs=1); sin/cos use ACT Sin with explicit round-to-nearest range reduction.
"""
import os
import sys

import numpy as np

for _p in ("/opt/trn_rl_repo", "/root/.axon_site/_ro/trn_rl_repo"):
    if os.path.isdir(_p) and _p not in sys.path:
        sys.path.append(_p)

import concourse.bacc as bacc
import concourse.mybir as mybir
import concourse.tile as tile
from concourse.tile import add_dep_helper
from concourse.bass_utils import run_bass_kernel_spmd

F32 = mybir.dt.float32
I32 = mybir.dt.int32
AF = mybir.ActivationFunctionType
OP = mybir.AluOpType

NCORES = 8
N = 32
TWO_PI = float(2.0 * np.pi)
INV_2PI = float(1.0 / (2.0 * np.pi))

_PROG = None
_PERM = None


def _params_perm():
    global _PERM
    if _PERM is None:
        g = np.arange(2112)
        i, c = g // 66, g % 66
        _PERM = np.where(
            c == 0, i,
            np.where(c == 1, 32 + i,
                     np.where(c < 34, 64 + 32 * i + (c - 2),
                              1088 + 32 * i + (c - 34)))).astype(np.int64)
    return _PERM


def _w2_perm(inp):
    return np.asarray(inp["in_W2"], np.float32)[_params_perm()]


def _b2_perm(inp):
    return np.asarray(inp["in_b2"], np.float32)[_params_perm()]


def _cpg_f_eval(nc, sb, ps, y, k, cst, sinr_e, acw_e):
    """k = f(y) for the CPG ODE. y, k: [32,3] tiles (cols a, a_dot, ph).

    sinr_e/acw_e: persistent [N, N+1] tiles; column N holds (1.0, ifr[i]) so
    the fused multiply-accumulate yields phase_dots = ifr + sum_j(...) in one
    instruction."""
    eye, ones, cw, pb, ia, ifr = cst
    # a, ph -> rows (PE transpose via identity), then broadcast to [32,32]
    arow_ps = ps.tile([1, N], F32, tag="rowps")
    phrow_ps = ps.tile([1, N], F32, tag="rowps")
    nc.tensor.matmul(arow_ps[:], y[:, 0:1], eye, start=True, stop=True)
    nc.tensor.matmul(phrow_ps[:], y[:, 2:3], eye, start=True, stop=True)
    arow = sb.tile([1, N], F32, tag="arow")
    phrow = sb.tile([1, N], F32, tag="phrow")
    nc.vector.tensor_copy(arow[:], arow_ps[:])
    nc.vector.tensor_copy(phrow[:], phrow_ps[:])
    A_ps = ps.tile([N, N], F32, tag="bcps")
    P_ps = ps.tile([N, N], F32, tag="bcps")
    nc.tensor.matmul(A_ps[:], ones, arow[:], start=True, stop=True)
    nc.tensor.matmul(P_ps[:], ones, phrow[:], start=True, stop=True)
    # arg[i,j] = ph_j - ph_i - pb[i,j], range-reduced into [-pi, pi]
    arg = sb.tile([N, N], F32, tag="arg")
    nc.vector.scalar_tensor_tensor(
        arg[:], P_ps[:], y[:, 2:3], pb[:], OP.subtract, OP.subtract)
    ri = sb.tile([N, N], I32, tag="ri")
    nc.vector.tensor_scalar(ri[:], arg[:], INV_2PI, None, OP.mult)
    nf = sb.tile([N, N], F32, tag="nf")
    nc.vector.tensor_copy(nf[:], ri[:])
    red = sb.tile([N, N], F32, tag="red")
    nc.vector.scalar_tensor_tensor(red[:], nf[:], -TWO_PI, arg[:], OP.mult, OP.add)
    nc.scalar.activation(sinr_e[:, 0:N], red[:], AF.Sin)
    # phase_dots = sum_j a_j * cw[i,j] * sin(arg[i,j])  + ifr via column N
    nc.vector.tensor_tensor(acw_e[:, 0:N], A_ps[:], cw[:], OP.mult)
    junk = sb.tile([N, N + 1], F32, tag="junk")
    nc.vector.scalar_tensor_tensor(
        junk[:], sinr_e[:], 0.0, acw_e[:], OP.bypass, OP.mult, accum_out=k[:, 1:2]
    )
    # k[:,0] = a_dot;  k[:,2] = 1000*(250*(ia - a) - a_dot)
    nc.vector.tensor_copy(k[:, 0:1], y[:, 1:2])
    t2 = sb.tile([N, 1], F32, tag="t2")
    nc.vector.tensor_scalar(t2[:], y[:, 0:1], ia[:], -250.0, OP.subtract, OP.mult)
    nc.vector.tensor_scalar(k[:, 2:3], t2[:], y[:, 1:2], 1000.0, OP.subtract, OP.mult)


def _range_reduce(nc, sb, src_col, extra_quarter_turn, tagp):
    """r = src - 2*pi*round((src + off)/(2*pi)); off = pi/2 when
    extra_quarter_turn so that r + pi/2 lands in [-pi, pi]."""
    ri = sb.tile([N, 1], I32, tag=tagp + "ri")
    if extra_quarter_turn:
        nc.vector.tensor_scalar(ri[:], src_col, INV_2PI, 0.25, OP.mult, OP.add)
    else:
        nc.vector.tensor_scalar(ri[:], src_col, INV_2PI, None, OP.mult)
    nf = sb.tile([N, 1], F32, tag=tagp + "nf")
    nc.vector.tensor_copy(nf[:], ri[:])
    r = sb.tile([N, 1], F32, tag=tagp + "r")
    nc.vector.scalar_tensor_tensor(r[:], nf[:], -TWO_PI, src_col, OP.mult, OP.add)
    return r


def _build():
    nc = bacc.Bacc(None, target_bir_lowering=False, num_devices=NCORES)

    # ---- I/O declarations (per-core shards, host-prepared layouts) ----
    m128_d = nc.dram_tensor("misc128", [128, 61], F32, kind="ExternalInput")
    m32_d = nc.dram_tensor("misc32", [N, 165], F32, kind="ExternalInput")
    m1_d = nc.dram_tensor("misc1", [1, 361], F32, kind="ExternalInput")
    w0_d = [nc.dram_tensor(f"w0b{j}", [128, 16 * 128], F32, kind="ExternalInput") for j in range(4)]
    w0t_d = nc.dram_tensor("w0tb", [2, 512], F32, kind="ExternalInput")
    w1_d = [nc.dram_tensor(f"w1b{j}", [128, 4 * 512], F32, kind="ExternalInput") for j in range(8)]
    w2_d = [nc.dram_tensor(f"w2b{g}", [128, 32 * mw], F32, kind="ExternalInput")
            for g, mw in enumerate((128, 128, 8))]
    w3_d = nc.dram_tensor("w3t", [65, 2048], F32, kind="ExternalInput")
    w4_d = nc.dram_tensor("w4t", [128, 16 * 256], F32, kind="ExternalInput")
    w5_d = nc.dram_tensor("w5t", [128, 2 * 1024], F32, kind="ExternalInput")
    stout_d = nc.dram_tensor("state_out", [N, 3], F32, kind="ExternalOutput")
    out_d = nc.dram_tensor("out_part", [128, 8], F32, kind="ExternalOutput")

    with tile.TileContext(nc) as tc:
        with (
            tc.tile_pool(name="w", bufs=1) as wp,
            tc.tile_pool(name="act", bufs=1) as ap,
            tc.tile_pool(name="tmp", bufs=2) as sb,
            tc.tile_pool(name="ps", bufs=2, space="PSUM") as ps,
            tc.tile_pool(name="dram", bufs=1, space="DRAM") as dp,
        ):
            # ---- packed constant loads (3 DMAs on the scalar ring) ----
            m128 = ap.tile([128, 61], F32, tag="m128")
            nc.scalar.dma_start(m128[:], m128_d[:])
            m32 = ap.tile([N, 165], F32, tag="m32")
            nc.scalar.dma_start(m32[:], m32_d[:])
            m1 = ap.tile([1, 361], F32, tag="m1")
            nc.scalar.dma_start(m1[:], m1_d[:])
            xin = m128[:, 0:17]
            hc = m128[0:N, 17:24]
            b2t = m128[:, 24:27]
            b3t = m128[:, 27:29]
            b1f = m128[:, 29:61]
            eye = m32[:, 0:32]
            y1 = m32[:, 32:35]
            sel65a = m32[:, 35:100]
            sel65b = m32[:, 100:165]
            ones = m1[:, 0:32]
            e64 = m1[:, 32:97]
            b2row = m1[:, 97:361]
            warm = ap.tile([1, 1], F32, tag="warm")
            nc.scalar.activation(warm[:], ones[0:1, 0:1], AF.Sin)

            # ---- weight streams (sync = HWDGE ring A, consumption order) ----
            w0 = [wp.tile([128, 16 * 128], F32, tag=f"w0_{j}", name=f"w0_{j}") for j in range(4)]
            for j in range(4):
                nc.sync.dma_start(w0[j][:], w0_d[j][:])
            w0tb = ap.tile([2, 512], F32, tag="w0tb")
            nc.scalar.dma_start(w0tb[:], w0t_d[:])
            w1 = [wp.tile([128, 4 * 512], F32, tag=f"w1_{j}", name=f"w1_{j}") for j in range(8)]
            for j in range(8):
                nc.sync.dma_start(w1[j][:], w1_d[j][:])
            w2 = [wp.tile([128, 32 * mw], F32, tag=f"w2_{g}", name=f"w2_{g}")
                  for g, mw in enumerate((128, 128, 8))]
            for g in range(3):
                nc.sync.dma_start(w2[g][:], w2_d[g][:])
            w3t = wp.tile([65, 2048], F32, tag="w3t")
            nc.sync.dma_start(w3t[:], w3_d[:])
            w4t = wp.tile([128, 16 * 256], F32, tag="w4t")
            w5t = wp.tile([128, 2 * 1024], F32, tag="w5t")

            # ---- L0: h0 = relu(W0 @ [x; t; 1]) (bias folded), 512 rows ----
            psum0 = ps.tile([128, 4], F32, tag="mm")
            for mj in range(4):
                for kc in range(16):
                    nc.tensor.matmul(
                        psum0[:, mj:mj + 1],
                        w0[mj][:, kc * 128:(kc + 1) * 128],
                        xin[:, kc:kc + 1],
                        start=(kc == 0), stop=False,
                    )
                # k rows 2048 (timestep) and 2049 (bias): xin[0:2, 16] = [t, 1]
                nc.tensor.matmul(
                    psum0[:, mj:mj + 1],
                    w0tb[:, mj * 128:(mj + 1) * 128],
                    xin[0:2, 16:17],
                    start=False, stop=True,
                )
            h0 = ap.tile([128, 4], F32, tag="h0")
            nc.vector.tensor_scalar_max(h0[:], psum0[:], 0.0)

            # ---- L1 partial: W1[:, cols_c] @ h0_c  (4096 partials) ----
            psum1 = ps.tile([128, 32], F32, tag="mm")
            for mj in range(32):
                blk, ml = mj // 4, (mj % 4) * 128
                for kc in range(4):
                    nc.tensor.matmul(
                        psum1[:, mj:mj + 1],
                        w1[blk][:, kc * 512 + ml: kc * 512 + ml + 128],
                        h0[:, kc:kc + 1],
                        start=(kc == 0), stop=(kc == 3),
                    )
            h1p = ap.tile([128, 32], F32, tag="h1p")
            nc.vector.tensor_copy(h1p[:], psum1[:])

            # ---- AllGather partials; local reduce (+bias as 9th slot) ----
            bounce1_in = dp.tile([128, 32], F32, tag="bc1i")
            bounce1_out = dp.tile([NCORES, 128, 32], F32, tag="bc1o")
            b1w = nc.scalar.dma_start(bounce1_in[:], h1p[:])
            # Out-MLP L1/L2 weights are needed only after the CPG; gating
            # them on the bounce write keeps the first 35us of HBM bandwidth
            # exclusively for the critical W0/W1 stream, then streams them
            # during the AllGather/L2 window.
            dw4 = nc.scalar.dma_start(w4t[:], w4_d[:])
            dw5 = nc.scalar.dma_start(w5t[:], w5_d[:])
            add_dep_helper(dw4.ins, b1w.ins, reason="defer w4t behind h1p bounce")
            add_dep_helper(dw5.ins, b1w.ins, reason="defer w5t behind h1p bounce")
            nc.gpsimd.collective_compute(
                "AllGather", OP.bypass, replica_groups=[list(range(NCORES))],
                ins=[bounce1_in.opt()], outs=[bounce1_out.opt()],
            )
            p8 = ap.tile([128, 9 * 32], F32, tag="p8")
            nc.scalar.dma_start(
                p8[:, 0:256].rearrange("p (r j) -> p r j", r=NCORES),
                bounce1_out[:].rearrange("r p j -> p r j"),
            )
            nc.vector.tensor_copy(p8[:, 256:288], b1f)
            h1s = ap.tile([128, 32], F32, tag="h1s")
            nc.vector.tensor_reduce(
                h1s[:], p8[:].rearrange("p (r j) -> p j r", r=9),
                axis=mybir.AxisListType.X, op=OP.add,
            )
            h1 = ap.tile([128, 32], F32, tag="h1")
            nc.vector.tensor_scalar_max(h1[:], h1s[:], 0.0)

            # ---- L2: params chunk = W2_c @ h1 + b2_c (264 rows); fp32
            # matmul moves at 4 cycles/column, so the N=1 column form (cost =
            # LDWEIGHTS only) beats a weight-moving row form here. ----
            psum2 = ps.tile([128, 3], F32, tag="mm")
            for g, mw in enumerate((128, 128, 8)):
                for kc in range(32):
                    nc.tensor.matmul(
                        psum2[0:mw, g:g + 1],
                        w2[g][:, kc * mw:(kc + 1) * mw],
                        h1[:, kc:kc + 1],
                        start=(kc == 0), stop=(kc == 31),
                    )
            params_sb = ap.tile([128, 3], F32, tag="params")
            nc.vector.tensor_add(params_sb[:], psum2[:], b2t)

            # ---- AllGather params -> replicated [ia, ifr, cw, pb] ----
            bounce2_in = dp.tile([264, 1], F32, tag="bc2i")
            bounce2_out = dp.tile([NCORES * 264, 1], F32, tag="bc2o")
            nc.scalar.dma_start(
                bounce2_in[0:256, :].rearrange("(j p) o -> p (j o)", j=2),
                params_sb[:, 0:2])
            nc.scalar.dma_start(bounce2_in[256:264, :], params_sb[0:8, 2:3])
            nc.gpsimd.collective_compute(
                "AllGather", OP.bypass, replica_groups=[list(range(NCORES))],
                ins=[bounce2_in.opt()], outs=[bounce2_out.opt()],
            )
            # host permuted W2/b2 so the gathered params are exactly a
            # row-major [32, 66] = [ia | ifr | cw row | pb row] tile
            pt = ap.tile([N, 66], F32, tag="pt")
            nc.scalar.dma_start(
                pt[:], bounce2_out[:].rearrange("(i c) o -> i (c o)", c=66))
            ia = pt[:, 0:1]
            ifr = pt[:, 1:2]
            cw = pt[:, 2:34]
            pb = pt[:, 34:66]

            # ---- CPG RK4 (3/8 rule), replicated.  Stage temps are arranged
            # so only ONE DVE op sits serially between f-evals. ----
            cst = (eye, ones, cw, pb, ia, ifr)
            h_c = hc[:, 0:1]
            h3_c = hc[:, 1:2]
            nh3_c = hc[:, 2:3]
            nh_c = hc[:, 3:4]
            h8_c = hc[:, 4:5]
            h38_c = hc[:, 5:6]
            k1 = ap.tile([N, 3], F32, tag="k1")
            k2 = ap.tile([N, 3], F32, tag="k2")
            k3 = ap.tile([N, 3], F32, tag="k3")
            k4 = ap.tile([N, 3], F32, tag="k4")
            sinr_es = [ap.tile([N, N + 1], F32, tag=f"sinr{e}", name=f"sinr{e}")
                       for e in range(4)]
            acw_es = [ap.tile([N, N + 1], F32, tag=f"acw{e}", name=f"acw{e}")
                      for e in range(4)]
            for e in range(4):
                nc.vector.memset(sinr_es[e][:, N:N + 1], 1.0)
                nc.vector.tensor_copy(acw_es[e][:, N:N + 1], ifr)
            _cpg_f_eval(nc, sb, ps, y1, k1, cst, sinr_es[0], acw_es[0])
            y2 = ap.tile([N, 3], F32, tag="y2")
            nc.vector.scalar_tensor_tensor(y2[:], k1[:], h3_c, y1, OP.mult, OP.add)
            # temps that only need k1 (overlap with eval 2)
            t13 = ap.tile([N, 3], F32, tag="t13")
            t15 = ap.tile([N, 3], F32, tag="t15")
            acc1 = ap.tile([N, 3], F32, tag="acc1")
            nc.vector.scalar_tensor_tensor(t13[:], k1[:], nh3_c, y1, OP.mult, OP.add)
            nc.vector.scalar_tensor_tensor(t15[:], k1[:], h_c, y1, OP.mult, OP.add)
            nc.vector.scalar_tensor_tensor(acc1[:], k1[:], h8_c, y1, OP.mult, OP.add)
            _cpg_f_eval(nc, sb, ps, y2, k2, cst, sinr_es[1], acw_es[1])
            y3 = ap.tile([N, 3], F32, tag="y3")
            nc.vector.scalar_tensor_tensor(y3[:], k2[:], h_c, t13[:], OP.mult, OP.add)
            t14 = ap.tile([N, 3], F32, tag="t14")
            acc2 = ap.tile([N, 3], F32, tag="acc2")
            nc.vector.scalar_tensor_tensor(t14[:], k2[:], nh_c, t15[:], OP.mult, OP.add)
            nc.vector.scalar_tensor_tensor(acc2[:], k2[:], h38_c, acc1[:], OP.mult, OP.add)
            _cpg_f_eval(nc, sb, ps, y3, k3, cst, sinr_es[2], acw_es[2])
            y4 = ap.tile([N, 3], F32, tag="y4")
            nc.vector.scalar_tensor_tensor(y4[:], k3[:], h_c, t14[:], OP.mult, OP.add)
            acc3 = ap.tile([N, 3], F32, tag="acc3")
            nc.vector.scalar_tensor_tensor(acc3[:], k3[:], h38_c, acc2[:], OP.mult, OP.add)
            _cpg_f_eval(nc, sb, ps, y4, k4, cst, sinr_es[3], acw_es[3])
            ynew = ap.tile([N, 3], F32, tag="ynew")
            nc.vector.scalar_tensor_tensor(ynew[:], k4[:], h8_c, acc3[:], OP.mult, OP.add)
            nc.scalar.dma_start(stout_d[:], ynew[:])

            # cpg_out = [a*cos(ph), a*sin(ph)]; reference reads ph from
            # new_state[n:2n], i.e. column 1 of ynew.
            rs = _range_reduce(nc, sb, ynew[:, 1:2], False, "rs")
            rc = _range_reduce(nc, sb, ynew[:, 1:2], True, "rc")
            sin_ph = ap.tile([N, 1], F32, tag="sinph")
            cos_ph = ap.tile([N, 1], F32, tag="cosph")
            nc.scalar.activation(sin_ph[:], rs[:], AF.Sin)
            nc.scalar.activation(cos_ph[:], rc[:], AF.Sin, bias=hc[:, 6:7])
            ac_col = ap.tile([N, 1], F32, tag="accol")
            as_col = ap.tile([N, 1], F32, tag="ascol")
            nc.vector.tensor_mul(ac_col[:], ynew[:, 0:1], cos_ph[:])
            nc.vector.tensor_mul(as_col[:], ynew[:, 0:1], sin_ph[:])

            # pack cpg_out (+bias 1.0) into a [65,1] column via PE selects
            cpg_ps = ps.tile([65, 1], F32, tag="mm")
            nc.tensor.matmul(cpg_ps[:], sel65a, ac_col[:], start=True, stop=False)
            nc.tensor.matmul(cpg_ps[:], sel65b, as_col[:], start=False, stop=False)
            nc.tensor.matmul(cpg_ps[:], e64, ones[0:1, 0:1], start=False, stop=True)
            cpg65 = ap.tile([65, 1], F32, tag="cpg65")
            nc.vector.tensor_copy(cpg65[:], cpg_ps[:])

            # ---- out-MLP L0 (replicated): g0 = relu(W0o @ cpg_out + b0o) ----
            psum3 = ps.tile([128, 16], F32, tag="mm")
            for mj in range(16):
                nc.tensor.matmul(psum3[:, mj:mj + 1],
                                 w3t[:, mj * 128:(mj + 1) * 128], cpg65[:],
                                 start=True, stop=True)
            g0 = ap.tile([128, 16], F32, tag="g0")
            nc.vector.tensor_scalar_max(g0[:], psum3[:], 0.0)

            # ---- out-MLP L1 (row-sharded): g1_c = relu(W1o_c @ g0 + b1o_c) ----
            psum4 = ps.tile([128, 2], F32, tag="mm")
            for mj in range(2):
                for kc in range(16):
                    nc.tensor.matmul(
                        psum4[:, mj:mj + 1],
                        w4t[:, kc * 256 + mj * 128: kc * 256 + mj * 128 + 128],
                        g0[:, kc:kc + 1],
                        start=(kc == 0), stop=(kc == 15),
                    )
            g1s = ap.tile([128, 2], F32, tag="g1s")
            nc.vector.tensor_add(g1s[:], psum4[:], b3t)
            g1c = ap.tile([128, 2], F32, tag="g1c")
            nc.vector.tensor_scalar_max(g1c[:], g1s[:], 0.0)

            # ---- out-MLP L2 (column-sharded): partial out[1024] ----
            psum5 = ps.tile([128, 8], F32, tag="mm")
            for mj in range(8):
                for kc in range(2):
                    nc.tensor.matmul(
                        psum5[:, mj:mj + 1],
                        w5t[:, kc * 1024 + mj * 128: kc * 1024 + mj * 128 + 128],
                        g1c[:, kc:kc + 1],
                        start=(kc == 0), stop=(kc == 1),
                    )
            out_sb = ap.tile([128, 8], F32, tag="outsb")
            nc.vector.tensor_copy(out_sb[:], psum5[:])
            nc.scalar.dma_start(out_d[:], out_sb[:])

    nc.compile()
    return nc


def _prep_core(c, inp):
    t = float(np.asarray(inp["timestep"], np.float32))
    x = np.asarray(inp["x"], np.float32)
    state = np.asarray(inp["state"], np.float32)

    x_aug = np.zeros(2176, np.float32)
    x_aug[:2048] = x
    x_aug[2048] = t
    x_aug[2049] = 1.0
    m = {}
    m128 = np.zeros((128, 61), np.float32)
    m128[:, 0:17] = x_aug.reshape(17, 128).T
    m128[:, 17:24] = np.array([t, t / 3, -t / 3, -t, t / 8, 3 * t / 8, np.pi / 2],
                              np.float32)[None, :]
    m32 = np.zeros((N, 165), np.float32)
    m32[:, 0:32] = np.eye(N, dtype=np.float32)
    m32[:, 32:35] = state.reshape(3, N).T
    m32[:, 35:100][:, 0:32] = np.eye(N, dtype=np.float32)
    m32[:, 100:165][:, 32:64] = np.eye(N, dtype=np.float32)
    m1 = np.zeros((1, 361), np.float32)
    m1[0, 0:32] = 1.0
    m1[0, 96] = 1.0
    m["misc128"] = m128
    m["misc32"] = m32
    m["misc1"] = m1

    R0 = slice(512 * c, 512 * c + 512)
    W0c = np.asarray(inp["in_W0"], np.float32)[R0]
    for j in range(4):
        blk = W0c[128 * j:128 * (j + 1), 0:2048]
        m[f"w0b{j}"] = np.ascontiguousarray(
            blk.reshape(128, 16, 128).transpose(2, 1, 0).reshape(128, 16 * 128))
    w0tb = np.empty((2, 512), np.float32)
    w0tb[0] = W0c[:, 2048]
    w0tb[1] = inp["in_b0"][R0]
    m["w0tb"] = w0tb

    W1c = inp["in_W1"][:, 512 * c:512 * (c + 1)]
    for j in range(8):
        blk = W1c[512 * j:512 * (j + 1)]
        m[f"w1b{j}"] = np.ascontiguousarray(
            blk.reshape(512, 4, 128).transpose(2, 1, 0).reshape(128, 4 * 512))
    m128[:, 29:61] = inp["in_b1"].reshape(32, 128).T

    R2 = slice(264 * c, 264 * (c + 1))
    W2c = _w2_perm(inp)[R2]
    b2c = _b2_perm(inp)[R2]
    for g, (mo, mw) in enumerate(((0, 128), (128, 128), (256, 8))):
        blk = W2c[mo:mo + mw]
        m[f"w2b{g}"] = np.ascontiguousarray(
            blk.reshape(mw, 32, 128).transpose(2, 1, 0).reshape(128, 32 * mw))
    m128[:, 24] = b2c[0:128]
    m128[:, 25] = b2c[128:256]
    m128[0:8, 26] = b2c[256:264]

    m["w3t"] = np.ascontiguousarray(np.concatenate(
        [inp["out_W0"][:, 0:32].T, inp["out_W0"][:, 32:64].T,
         np.asarray(inp["out_b0"], np.float32)[None, :]], axis=0))

    W1oc = inp["out_W1"][256 * c:256 * (c + 1), :]
    m["w4t"] = np.ascontiguousarray(
        W1oc.reshape(256, 16, 128).transpose(2, 1, 0).reshape(128, 16 * 256))
    m128[:, 27:29] = np.asarray(
        inp["out_b1"], np.float32)[256 * c:256 * (c + 1)].reshape(2, 128).T

    W2oc = inp["out_W2"][:, 256 * c:256 * (c + 1)]
    m["w5t"] = np.ascontiguousarray(
        W2oc.reshape(1024, 2, 128).transpose(2, 1, 0).reshape(128, 2 * 1024))
    return m


def kernel(**inputs):
    global _PROG
    if _PROG is None:
        _PROG = _build()
    in_maps = [_prep_core(c, inputs) for c in range(NCORES)]
    res = run_bass_kernel_spmd(_PROG, in_maps, core_ids=list(range(NCORES)))
    state_out = np.ascontiguousarray(res.results[0]["state_out"].T).reshape(96)
    acc = res.results[0]["out_part"].astype(np.float32).copy()
    for c in range(1, NCORES):
        acc += res.results[c]["out_part"]
    out = np.ascontiguousarray(acc.T).reshape(1024) + np.asarray(inputs["out_b2"], np.float32)
    return (state_out.astype(np.float32), out.astype(np.float32))
